# revision 8
# baseline (speedup 1.0000x reference)
"""Trainium2 Bass kernel for multiresolution hash-grid encoding (Instant-NGP style).

Contract: kernel(x01, tables) -> [N, 16] float32, computed on 8 NeuronCores.

Strategy ("mega-row", one gather per point):
  The SWDGE indirect-DMA ucode reads exactly ONE offset per partition per
  instruction (~1.4us each), so instructions = points/128 * gathers_per_point.
  We make gathers_per_point = 1: a single "carrier" grid at lvl5 resolution
  (115^3 cells) whose 1280B row holds data for ALL 8 levels:
    - lvl5 cube (8 corners x 2 feats, fp16)          -> exact trilinear lerp
    - lvl1 cube (23 | 115, cell1 = cell5 coords //5) -> exact trilinear lerp
    - lvls 0,2,3,4: 3^3-vertex patch (R_l < 115: the carrier cell pins the
      lvl-l cell to {m, m+1} per dim; patch = vertices m..m+2)
    - lvl6: 4^3 patch (172/115 < 2 -> cells m..m+2, vertices m..m+3)
    - lvl7: 5^3 patch (256/115 < 3 -> cells m..m+3, vertices m..m+4)
  The device recomputes m = floor(cell5 * (R_l/115)) with the same f32 math
  as the host, takes b = floor(x*R_l) - m in {0..K}, and evaluates the lerp
  over the patch with per-dim node weights (one-hot shift of [1-f, f] by b),
  all in fp16 on DVE. Host reassembles [P,TP,8,2] fp16 -> [N,16] f32.
"""
import math
import numpy as np

NUM_LEVELS = 8
FEATS = 2
TABLE_SIZE = 2 ** 18
MIN_RES = 16
MAX_RES = 256
GROWTH = math.exp(math.log(MAX_RES / MIN_RES) / (NUM_LEVELS - 1))
P1, P2, P3 = 1540863, 1256879, 1957123
RES = [int(math.floor(MIN_RES * GROWTH ** l + 1e-6)) for l in range(NUM_LEVELS)]

N_CORES = 8
P = 128          # SBUF partitions
TP = 2048        # points per partition per core
T_B = 64         # points per partition per batch
PTS_PER_CORE = P * TP          # 262144
N_PAD = N_CORES * PTS_PER_CORE  # 2097152

CARRIER = 5
RC = RES[CARRIER]               # 115
# row layout, in halfs (fp16): level -> (offset, kind, nodes)
# kind: 'cube' = 16-half corner cube; 'patch' = nodes^3 x 2 halfs
ROW_SPEC = {
    5: (0, "cube", 0),
    1: (16, "cube", 0),
    0: (32, "patch", 3),
    2: (86, "patch", 3),
    3: (140, "patch", 3),
    4: (194, "patch", 3),
    6: (248, "patch", 4),
    7: (376, "patch", 5),
}
ROW_H = 640                     # 626 used, padded to 640 halfs = 1280 B


def _vertex_emb(tables: np.ndarray, l: int) -> np.ndarray:
    """fp16 vertex embeddings [R+1, R+1, R+1, 2] for level l."""
    R = RES[l]
    n = R + 1
    mask = TABLE_SIZE - 1
    vx = np.arange(n, dtype=np.int64) * P1
    vy = np.arange(n, dtype=np.int64) * P2
    vz = np.arange(n, dtype=np.int64) * P3
    idx = (vx[:, None, None] ^ vy[None, :, None] ^ vz[None, None, :]) & mask
    return tables[l][idx].astype(np.float16)


def _cube_from_vemb(vemb: np.ndarray) -> np.ndarray:
    """[R,R,R,8,2] fp16 cube rows from [R+1,...] vertices."""
    R = vemb.shape[0] - 1
    cube = np.empty((R, R, R, 8, FEATS), dtype=np.float16)
    e = 0
    for dx in (0, 1):
        for dy in (0, 1):
            for dz in (0, 1):
                cube[:, :, :, e, :] = vemb[dx:dx + R, dy:dy + R, dz:dz + R]
                e += 1
    return cube


def _patch_base(l: int) -> np.ndarray:
    """m[c] = floor(f32(c) * f32(R_l/RC)) for c in 0..RC-1 (f32 math, as device)."""
    ratio = np.float32(float(RES[l]) / float(RC))
    c = np.arange(RC, dtype=np.float32)
    return np.floor(c * ratio).astype(np.int64)


def _build_mega_grid(tables: np.ndarray) -> np.ndarray:
    mega = np.zeros((RC ** 3, ROW_H), dtype=np.float16)

    for l, (off, kind, nodes) in ROW_SPEC.items():
        vemb = _vertex_emb(tables, l)
        if kind == "cube":
            if l == CARRIER:
                cube = _cube_from_vemb(vemb).reshape(RC ** 3, 16)
            else:  # lvl1, 23*5 = 115: cell1 = cell5 // 5
                s = RC // RES[l]
                c1 = _cube_from_vemb(vemb)  # [23,23,23,8,2]
                c1 = np.repeat(np.repeat(np.repeat(c1, s, 0), s, 1), s, 2)
                cube = c1.reshape(RC ** 3, 16)
            mega[:, off:off + 16] = cube
            del cube
        else:
            R = RES[l]
            m = _patch_base(l)                       # [RC]
            ar = np.arange(nodes, dtype=np.int64)
            ix = np.minimum(m[:, None] + ar[None, :], R)  # [RC, nodes]
            blk = vemb[
                ix[:, None, None, :, None, None],
                ix[None, :, None, None, :, None],
                ix[None, None, :, None, None, :],
            ]  # [RC, RC, RC, n, n, n, 2]
            mega[:, off:off + nodes ** 3 * 2] = blk.reshape(RC ** 3, nodes ** 3 * 2)
            del blk
        del vemb
    return mega


def _build_program():
    import concourse.bass as bass
    import concourse.bacc as bacc
    import concourse.tile as tile
    from concourse import mybir

    f32 = mybir.dt.float32
    f16 = mybir.dt.float16
    i32 = mybir.dt.int32
    Alu = mybir.AluOpType

    nc = bacc.Bacc("TRN2", target_bir_lowering=False, debug=False)
    x_ext = nc.dram_tensor("x", [P, TP, 3], f32, kind="ExternalInput")
    g_ext = nc.dram_tensor("g", [RC ** 3, ROW_H], f16, kind="ExternalInput")
    out_ext = nc.dram_tensor("out", [P, TP, NUM_LEVELS, FEATS], f16,
                             kind="ExternalOutput")

    n_batches = TP // T_B
    T = T_B
    XHI = 1.0 - 2.0 ** -24
    XLO = 2.0 ** -18

    def addr(nc, coord, coordt, xt, R, tag, itag=None):
        """floor + frac for resolution R: returns (i0f f32 [P,3T], frach f16)."""
        if itag is None:
            itag = f"i0f_{tag}"
        icst = coordt.tile([P, T * 3], i32, tag=f"icst", name=f"icst_{tag}")
        nc.vector.tensor_scalar(out=icst[:], in0=xt[:], scalar1=float(R),
                                scalar2=None, op0=Alu.mult)
        ipool = coord if tag == "c5" else coordt
        i0f = ipool.tile([P, T * 3], f32, tag=itag, name=f"i0f_{tag}")
        nc.vector.tensor_copy(out=i0f[:], in_=icst[:])
        up = coordt.tile([P, T * 3], f32, tag="up", name=f"up_{tag}")
        nc.vector.scalar_tensor_tensor(out=up[:], in0=xt[:], scalar=float(R),
                                       in1=i0f[:], op0=Alu.mult, op1=Alu.is_lt)
        nc.vector.tensor_tensor(out=i0f[:], in0=i0f[:], in1=up[:],
                                op=Alu.subtract)
        frach = coordt.tile([P, T * 3], f16, tag=f"frach", name=f"frach_{tag}")
        nc.vector.scalar_tensor_tensor(out=frach[:], in0=xt[:], scalar=float(R),
                                       in1=i0f[:], op0=Alu.mult, op1=Alu.subtract)
        return i0f, frach

    def cube_lerp(nc, lerpp, cube6, frach, ol_view, tag):
        """Standard trilinear lerp of an [P, T, 8, 2]-view cube (a z f order).

        cube6: view [P, T, 4, 2(z), 2(f)] after caller rearrange; here we take
        the flat [P, T, 16] slice view and rearrange locally.
        """
        cz = cube6.rearrange("p t (a z f) -> p t a z f", a=4, z=2, f=2)
        e0, e1 = cz[:, :, :, 0, :], cz[:, :, :, 1, :]
        f3 = frach[:].rearrange("p (t c) -> p t c", c=3)
        fx, fy, fz = f3[:, :, 0], f3[:, :, 1], f3[:, :, 2]

        az = lerpp.tile([P, T * 8], f16, tag="az", name=f"az_{tag}")
        az4 = az[:].rearrange("p (t a f) -> p t a f", a=4, f=2)
        dz = lerpp.tile([P, T * 8], f16, tag="dz", name=f"dz_{tag}")
        dz4 = dz[:].rearrange("p (t a f) -> p t a f", a=4, f=2)
        nc.vector.tensor_tensor(out=dz4, in0=e1, in1=e0, op=Alu.subtract)
        nc.vector.tensor_tensor(out=dz4, in0=dz4,
                                in1=fz.to_broadcast([P, T, 4, 2]), op=Alu.mult)
        nc.vector.tensor_tensor(out=az4, in0=dz4, in1=e0, op=Alu.add)

        ay = lerpp.tile([P, T * 4], f16, tag="ay", name=f"ay_{tag}")
        ay4 = ay[:].rearrange("p (t a f) -> p t a f", a=2, f=2)
        azy = az[:].rearrange("p (t a y f) -> p t a y f", a=2, y=2, f=2)
        y0, y1 = azy[:, :, :, 0, :], azy[:, :, :, 1, :]
        dy_ = lerpp.tile([P, T * 4], f16, tag="dy", name=f"dy_{tag}")
        dy4 = dy_[:].rearrange("p (t a f) -> p t a f", a=2, f=2)
        nc.vector.tensor_tensor(out=dy4, in0=y1, in1=y0, op=Alu.subtract)
        nc.vector.tensor_tensor(out=dy4, in0=dy4,
                                in1=fy.to_broadcast([P, T, 2, 2]), op=Alu.mult)
        nc.vector.tensor_tensor(out=ay4, in0=dy4, in1=y0, op=Alu.add)

        ayx = ay[:].rearrange("p (t x f) -> p t x f", x=2, f=2)
        x0, x1 = ayx[:, :, 0, :], ayx[:, :, 1, :]
        dx_ = lerpp.tile([P, T * 2], f16, tag="dx", name=f"dx_{tag}")
        dx2 = dx_[:].rearrange("p (t f) -> p t f", f=2)
        nc.vector.tensor_tensor(out=dx2, in0=x1, in1=x0, op=Alu.subtract)
        nc.vector.tensor_tensor(out=dx2, in0=dx2,
                                in1=fx.to_broadcast([P, T, 2]), op=Alu.mult)
        nc.vector.tensor_tensor(out=ol_view, in0=dx2, in1=x0, op=Alu.add)

    def patch_level(nc, coord, coordt, lerpp, l, nodes, xt, i0f5, mega_v, off,
                    ol_view):
        """Patch lerp for level l with `nodes` nodes/dim."""
        R = RES[l]
        ratio = float(R) / float(RC)
        i0f, frach = addr(nc, coord, coordt, xt, R, f"p{l}", itag="i0f_patch")
        # m = floor(i0f5 * ratio) with the same robust f32 math
        mi = coordt.tile([P, T * 3], i32, tag="mi", name=f"mi_{l}")
        nc.vector.tensor_scalar(out=mi[:], in0=i0f5[:], scalar1=ratio,
                                scalar2=None, op0=Alu.mult)
        mf = coordt.tile([P, T * 3], f32, tag="mf", name=f"mf_{l}")
        nc.vector.tensor_copy(out=mf[:], in_=mi[:])
        upm = coordt.tile([P, T * 3], f32, tag="up", name=f"upm_{l}")
        nc.vector.scalar_tensor_tensor(out=upm[:], in0=i0f5[:], scalar=ratio,
                                       in1=mf[:], op0=Alu.mult, op1=Alu.is_lt)
        nc.vector.tensor_tensor(out=mf[:], in0=mf[:], in1=upm[:],
                                op=Alu.subtract)
        b = coordt.tile([P, T * 3], f32, tag="b", name=f"b_{l}")
        nc.vector.tensor_tensor(out=b[:], in0=i0f[:], in1=mf[:], op=Alu.subtract)

        # node weights w_j [P, 3T] f16: shift of [1-f, f] by b (cells = nodes-1)
        # e_j = (b == j) f32; fe_j = frac * e_j;
        # w_0 = e_0 - fe_0; w_j = e_j - fe_j + fe_{j-1}; w_{n-1} = fe_{n-2}
        ncell = nodes - 1
        fracf = coordt.tile([P, T * 3], f32, tag="fracf", name=f"fracf_{l}")
        nc.vector.tensor_copy(out=fracf[:], in_=frach[:])
        es = []
        for j in range(ncell):
            e = coordt.tile([P, T * 3], f32, tag=f"e{j}", name=f"e{j}_{l}")
            nc.vector.tensor_scalar(out=e[:], in0=b[:], scalar1=float(j),
                                    scalar2=None, op0=Alu.is_equal)
            es.append(e)
        fes = []
        for j in range(ncell):
            fe = coordt.tile([P, T * 3], f32, tag=f"fe{j}", name=f"fe{j}_{l}")
            nc.vector.tensor_tensor(out=fe[:], in0=fracf[:], in1=es[j][:],
                                    op=Alu.mult)
            fes.append(fe)
        ws = []
        for j in range(nodes):
            w = coordt.tile([P, T * 3], f16, tag=f"w{j}", name=f"w{j}_{l}")
            if j == 0:
                nc.vector.tensor_tensor(out=w[:], in0=es[0][:], in1=fes[0][:],
                                        op=Alu.subtract)
            elif j < ncell:
                tmp = coordt.tile([P, T * 3], f32, tag="wtmp", name=f"wtmp{j}_{l}")
                nc.vector.tensor_tensor(out=tmp[:], in0=es[j][:], in1=fes[j][:],
                                        op=Alu.subtract)
                nc.vector.tensor_tensor(out=w[:], in0=tmp[:], in1=fes[j - 1][:],
                                        op=Alu.add)
            else:
                nc.vector.tensor_copy(out=w[:], in_=fes[ncell - 1][:])
            ws.append(w)

        def wdim(j, d):
            return ws[j][:].rearrange("p (t c) -> p t c", c=3)[:, :, d]

        n = nodes
        # z reduce -> [P, T, n*n, 2]; (a b) merged so APs stay <= 3 free dims
        pv = mega_v[:, :, off:off + n ** 3 * 2].rearrange(
            "p t (ab c f) -> p t ab c f", ab=n * n, c=n, f=2)
        accz = lerpp.tile([P, T * n * n * 2], f16, tag="accz", name=f"accz_{l}")
        azm = accz[:].rearrange("p (t ab f) -> p t ab f", ab=n * n, f=2)
        tmpz = lerpp.tile([P, T * n * n * 2], f16, tag="tmpz", name=f"tmpz_{l}")
        tzm = tmpz[:].rearrange("p (t ab f) -> p t ab f", ab=n * n, f=2)
        for k in range(n):
            dst = azm if k == 0 else tzm
            nc.vector.tensor_tensor(
                out=dst, in0=pv[:, :, :, k, :],
                in1=wdim(k, 2).to_broadcast([P, T, n * n, 2]), op=Alu.mult)
            if k > 0:
                nc.vector.tensor_tensor(out=accz[:], in0=accz[:], in1=tmpz[:],
                                        op=Alu.add)
        az6 = accz[:].rearrange("p (t a b f) -> p t a b f", a=n, b=n, f=2)
        # y reduce -> [P, T, n, 2]
        accy = lerpp.tile([P, T * n * 2], f16, tag="accy", name=f"accy_{l}")
        ay5 = accy[:].rearrange("p (t a f) -> p t a f", a=n, f=2)
        tmpy = lerpp.tile([P, T * n * 2], f16, tag="tmpy", name=f"tmpy_{l}")
        ty5 = tmpy[:].rearrange("p (t a f) -> p t a f", a=n, f=2)
        for k in range(n):
            dst = ay5 if k == 0 else ty5
            nc.vector.tensor_tensor(
                out=dst, in0=az6[:, :, :, k, :],
                in1=wdim(k, 1).to_broadcast([P, T, n, 2]), op=Alu.mult)
            if k > 0:
                nc.vector.tensor_tensor(out=ay5, in0=ay5, in1=ty5, op=Alu.add)
        # x reduce -> [P, T, 2] into ol_view
        tmpx = lerpp.tile([P, T * 2], f16, tag="tmpx", name=f"tmpx_{l}")
        tx3 = tmpx[:].rearrange("p (t f) -> p t f", f=2)
        for k in range(n):
            dst = ol_view if k == 0 else tx3
            nc.vector.tensor_tensor(
                out=dst, in0=ay5[:, :, k, :],
                in1=wdim(k, 0).to_broadcast([P, T, 2]), op=Alu.mult)
            if k > 0:
                nc.vector.tensor_tensor(out=ol_view, in0=ol_view, in1=tx3,
                                        op=Alu.add)

    with tile.TileContext(nc) as tc:
        with (
            tc.tile_pool(name="xp", bufs=2) as xp,
            tc.tile_pool(name="coord", bufs=2) as coord,
            tc.tile_pool(name="coordt", bufs=1) as coordt,
            tc.tile_pool(name="idxp", bufs=2) as idxp,
            tc.tile_pool(name="megap", bufs=2) as megap,
            tc.tile_pool(name="lerpp", bufs=1) as lerpp,
            tc.tile_pool(name="outp", bufs=2) as outp,
        ):
            for bi in range(n_batches):
                xt = xp.tile([P, T * 3], f32, tag="x", name="xt")
                nc.sync.dma_start(out=xt[:],
                                  in_=x_ext.ap()[:, bi * T:(bi + 1) * T, :])
                nc.vector.tensor_scalar(out=xt[:], in0=xt[:], scalar1=XHI,
                                        scalar2=XLO, op0=Alu.min, op1=Alu.max)

                # carrier address + flatten
                i0f5, frach5 = addr(nc, coord, coordt, xt, RC, "c5")
                i3 = i0f5[:].rearrange("p (t c) -> p t c", c=3)
                ix, iy, iz = i3[:, :, 0], i3[:, :, 1], i3[:, :, 2]
                cellf = coordt.tile([P, T], f32, tag="cellf", name="cellf")
                nc.vector.scalar_tensor_tensor(out=cellf[:], in0=ix,
                                               scalar=float(RC), in1=iy,
                                               op0=Alu.mult, op1=Alu.add)
                idx = idxp.tile([P, T], i32, tag="idx", name="idx")
                nc.vector.scalar_tensor_tensor(out=idx[:], in0=cellf[:],
                                               scalar=float(RC), in1=iz,
                                               op0=Alu.mult, op1=Alu.add)

                mega = megap.tile([P, T * ROW_H], f16, tag="mega", name="mega")
                for j in range(T):
                    nc.gpsimd.indirect_dma_start(
                        out=mega[:, j * ROW_H:(j + 1) * ROW_H],
                        out_offset=None,
                        in_=g_ext.ap(),
                        in_offset=bass.IndirectOffsetOnAxis(
                            ap=idx[:, j:j + 1], axis=0),
                    )
                mega_v = mega[:].rearrange("p (t h) -> p t h", h=ROW_H)

                ol = outp.tile([P, T * NUM_LEVELS * FEATS], f16, tag="ol",
                               name="ol")
                ol4 = ol[:].rearrange("p (t l f) -> p t l f", l=NUM_LEVELS,
                                      f=FEATS)

                # exact levels
                cube_lerp(nc, lerpp, mega_v[:, :, 0:16], frach5,
                          ol4[:, :, 5, :], "l5")
                i0f1, frach1 = addr(nc, coord, coordt, xt, RES[1], "c1")
                cube_lerp(nc, lerpp, mega_v[:, :, 16:32], frach1,
                          ol4[:, :, 1, :], "l1")

                # patched levels
                for l in (0, 2, 3, 4, 6, 7):
                    off, _, nodes = ROW_SPEC[l]
                    patch_level(nc, coord, coordt, lerpp, l, nodes, xt, i0f5,
                                mega_v, off, ol4[:, :, l, :])

                nc.sync.dma_start(
                    out=out_ext.ap()[:, bi * T:(bi + 1) * T, :, :],
                    in_=ol[:],
                )
    nc.compile()
    return nc


_PROGRAM_CACHE = {}


def kernel(x01: np.ndarray, tables: np.ndarray, _trace: bool = False,
           _tmpdir: str | None = None) -> np.ndarray:
    from concourse.bass_utils import run_bass_kernel_spmd

    N = x01.shape[0]
    assert N <= N_PAD, (N, N_PAD)

    mega = _build_mega_grid(np.asarray(tables, dtype=np.float32))

    xp = np.zeros((N_PAD, 3), dtype=np.float32)
    xp[:N] = np.asarray(x01, dtype=np.float32)

    key = "prog"
    if key not in _PROGRAM_CACHE:
        _PROGRAM_CACHE[key] = _build_program()
    nc = _PROGRAM_CACHE[key]

    in_maps = []
    for c in range(N_CORES):
        m = {"x": xp[c * PTS_PER_CORE:(c + 1) * PTS_PER_CORE].reshape(P, TP, 3),
             "g": mega}
        in_maps.append(m)

    res = run_bass_kernel_spmd(
        nc, in_maps, core_ids=list(range(N_CORES)),
        trace=_trace, tmpdir=_tmpdir,
    )

    # out per core: [P, TP, 8, 2] fp16 (point-major) -> [N, 16] f32
    parts = [r["out"].reshape(PTS_PER_CORE, NUM_LEVELS * FEATS)
             for r in res.results]
    out = np.concatenate(parts, axis=0).astype(np.float32)
    if _trace:
        kernel.last_exec_time_ns = res.exec_time_ns
        kernel.last_results = res
    return np.ascontiguousarray(out[:N])


# revision 9
# speedup vs baseline: 1.3207x; 1.3207x over previous
"""Trainium2 Bass kernel for multiresolution hash-grid encoding (Instant-NGP style).

Contract: kernel(x01, tables) -> [N, 16] float32, computed on 8 NeuronCores.

Strategy ("mega-row", one gather per point):
  The SWDGE indirect-DMA ucode reads exactly ONE offset per partition per
  instruction (~1.4us each), so instructions = points/128 * gathers_per_point.
  We make gathers_per_point = 1: a single "carrier" grid at lvl5 resolution
  (115^3 cells) whose 1280B row holds data for ALL 8 levels:
    - lvl5 cube (8 corners x 2 feats, fp16)          -> exact trilinear lerp
    - lvl1 cube (23 | 115, cell1 = cell5 coords //5) -> exact trilinear lerp
    - lvls 0,2,3,4: 3^3-vertex patch (R_l < 115: the carrier cell pins the
      lvl-l cell to {m, m+1} per dim; patch = vertices m..m+2)
    - lvl6: 4^3 patch (172/115 < 2 -> cells m..m+2, vertices m..m+3)
    - lvl7: 5^3 patch (256/115 < 3 -> cells m..m+3, vertices m..m+4)
  The device recomputes m = floor(cell5 * (R_l/115)) with the same f32 math
  as the host, takes b = floor(x*R_l) - m in {0..K}, and evaluates the lerp
  over the patch with per-dim node weights (one-hot shift of [1-f, f] by b),
  all in fp16 on DVE. Host reassembles [P,TP,8,2] fp16 -> [N,16] f32.
"""
import math
import numpy as np

NUM_LEVELS = 8
FEATS = 2
TABLE_SIZE = 2 ** 18
MIN_RES = 16
MAX_RES = 256
GROWTH = math.exp(math.log(MAX_RES / MIN_RES) / (NUM_LEVELS - 1))
P1, P2, P3 = 1540863, 1256879, 1957123
RES = [int(math.floor(MIN_RES * GROWTH ** l + 1e-6)) for l in range(NUM_LEVELS)]

N_CORES = 8
P = 128          # SBUF partitions
TP = 2048        # points per partition per core
T_B = 64         # points per partition per batch
PTS_PER_CORE = P * TP          # 262144
N_PAD = N_CORES * PTS_PER_CORE  # 2097152

CARRIER = 5
RC = RES[CARRIER]               # 115
# row layout, in halfs (fp16): level -> (offset, kind, nodes)
# kind: 'cube' = 16-half corner cube; 'patch' = nodes^3 x 2 halfs
ROW_SPEC = {
    5: (0, "cube", 0),
    1: (16, "cube", 0),
    0: (32, "patch", 3),
    2: (86, "patch", 3),
    3: (140, "patch", 3),
    4: (194, "patch", 3),
    6: (248, "patch", 4),
    7: (376, "patch", 5),
}
ROW_H = 640                     # 626 used, padded to 640 halfs = 1280 B


def _vertex_emb(tables: np.ndarray, l: int) -> np.ndarray:
    """fp16 vertex embeddings [R+1, R+1, R+1, 2] for level l."""
    R = RES[l]
    n = R + 1
    mask = TABLE_SIZE - 1
    vx = np.arange(n, dtype=np.int64) * P1
    vy = np.arange(n, dtype=np.int64) * P2
    vz = np.arange(n, dtype=np.int64) * P3
    idx = (vx[:, None, None] ^ vy[None, :, None] ^ vz[None, None, :]) & mask
    return tables[l][idx].astype(np.float16)


def _cube_from_vemb(vemb: np.ndarray) -> np.ndarray:
    """[R,R,R,8,2] fp16 cube rows from [R+1,...] vertices."""
    R = vemb.shape[0] - 1
    cube = np.empty((R, R, R, 8, FEATS), dtype=np.float16)
    e = 0
    for dx in (0, 1):
        for dy in (0, 1):
            for dz in (0, 1):
                cube[:, :, :, e, :] = vemb[dx:dx + R, dy:dy + R, dz:dz + R]
                e += 1
    return cube


def _patch_base(l: int) -> np.ndarray:
    """m[c] = floor(f32(c) * f32(R_l/RC)) for c in 0..RC-1 (f32 math, as device)."""
    ratio = np.float32(float(RES[l]) / float(RC))
    c = np.arange(RC, dtype=np.float32)
    return np.floor(c * ratio).astype(np.int64)


def _build_mega_grid(tables: np.ndarray) -> np.ndarray:
    mega = np.zeros((RC ** 3, ROW_H), dtype=np.float16)

    for l, (off, kind, nodes) in ROW_SPEC.items():
        vemb = _vertex_emb(tables, l)
        if kind == "cube":
            if l == CARRIER:
                cube = _cube_from_vemb(vemb).reshape(RC ** 3, 16)
            else:  # lvl1, 23*5 = 115: cell1 = cell5 // 5
                s = RC // RES[l]
                c1 = _cube_from_vemb(vemb)  # [23,23,23,8,2]
                c1 = np.repeat(np.repeat(np.repeat(c1, s, 0), s, 1), s, 2)
                cube = c1.reshape(RC ** 3, 16)
            mega[:, off:off + 16] = cube
            del cube
        else:
            R = RES[l]
            m = _patch_base(l)                       # [RC]
            ar = np.arange(nodes, dtype=np.int64)
            ix = np.minimum(m[:, None] + ar[None, :], R)  # [RC, nodes]
            blk = vemb[
                ix[:, None, None, :, None, None],
                ix[None, :, None, None, :, None],
                ix[None, None, :, None, None, :],
            ]  # [RC, RC, RC, n, n, n, 2]
            mega[:, off:off + nodes ** 3 * 2] = blk.reshape(RC ** 3, nodes ** 3 * 2)
            del blk
        del vemb
    return mega


def _build_program():
    import concourse.bass as bass
    import concourse.bacc as bacc
    import concourse.tile as tile
    from concourse import mybir

    f32 = mybir.dt.float32
    f16 = mybir.dt.float16
    i32 = mybir.dt.int32
    Alu = mybir.AluOpType

    nc = bacc.Bacc("TRN2", target_bir_lowering=False, debug=False)
    x_ext = nc.dram_tensor("x", [P, TP, 3], f32, kind="ExternalInput")
    g_ext = nc.dram_tensor("g", [RC ** 3, ROW_H], f16, kind="ExternalInput")
    out_ext = nc.dram_tensor("out", [P, TP, NUM_LEVELS, FEATS], f16,
                             kind="ExternalOutput")

    n_batches = TP // T_B
    T = T_B
    XHI = 1.0 - 2.0 ** -24
    XLO = 2.0 ** -18

    def addr(nc, coord, coordt, xt, R, tag, itag=None):
        """floor + frac for resolution R: returns (i0f f32 [P,3T], frach f16)."""
        if itag is None:
            itag = f"i0f_{tag}"
        tpool = coord if tag == "c5" else coordt
        sfx = "5" if tag == "c5" else ""
        icst = tpool.tile([P, T * 3], i32, tag=f"icst{sfx}", name=f"icst_{tag}")
        nc.vector.tensor_scalar(out=icst[:], in0=xt[:], scalar1=float(R),
                                scalar2=None, op0=Alu.mult)
        ipool = coord if tag == "c5" else coordt
        i0f = ipool.tile([P, T * 3], f32, tag=itag, name=f"i0f_{tag}")
        nc.vector.tensor_copy(out=i0f[:], in_=icst[:])
        up = tpool.tile([P, T * 3], f32, tag=f"up{sfx}", name=f"up_{tag}")
        nc.vector.scalar_tensor_tensor(out=up[:], in0=xt[:], scalar=float(R),
                                       in1=i0f[:], op0=Alu.mult, op1=Alu.is_lt)
        nc.vector.tensor_tensor(out=i0f[:], in0=i0f[:], in1=up[:],
                                op=Alu.subtract)
        frach = tpool.tile([P, T * 3], f16, tag=f"frach{sfx}", name=f"frach_{tag}")
        nc.vector.scalar_tensor_tensor(out=frach[:], in0=xt[:], scalar=float(R),
                                       in1=i0f[:], op0=Alu.mult, op1=Alu.subtract)
        return i0f, frach

    def cube_lerp(nc, lerpp, cube6, frach, ol_view, tag):
        """Standard trilinear lerp of an [P, T, 8, 2]-view cube (a z f order).

        cube6: view [P, T, 4, 2(z), 2(f)] after caller rearrange; here we take
        the flat [P, T, 16] slice view and rearrange locally.
        """
        cz = cube6.rearrange("p t (a z f) -> p t a z f", a=4, z=2, f=2)
        e0, e1 = cz[:, :, :, 0, :], cz[:, :, :, 1, :]
        f3 = frach[:].rearrange("p (t c) -> p t c", c=3)
        fx, fy, fz = f3[:, :, 0], f3[:, :, 1], f3[:, :, 2]

        az = lerpp.tile([P, T * 8], f16, tag="az", name=f"az_{tag}")
        az4 = az[:].rearrange("p (t a f) -> p t a f", a=4, f=2)
        dz = lerpp.tile([P, T * 8], f16, tag="dz", name=f"dz_{tag}")
        dz4 = dz[:].rearrange("p (t a f) -> p t a f", a=4, f=2)
        nc.vector.tensor_tensor(out=dz4, in0=e1, in1=e0, op=Alu.subtract)
        nc.vector.tensor_tensor(out=dz4, in0=dz4,
                                in1=fz.to_broadcast([P, T, 4, 2]), op=Alu.mult)
        nc.vector.tensor_tensor(out=az4, in0=dz4, in1=e0, op=Alu.add)

        ay = lerpp.tile([P, T * 4], f16, tag="ay", name=f"ay_{tag}")
        ay4 = ay[:].rearrange("p (t a f) -> p t a f", a=2, f=2)
        azy = az[:].rearrange("p (t a y f) -> p t a y f", a=2, y=2, f=2)
        y0, y1 = azy[:, :, :, 0, :], azy[:, :, :, 1, :]
        dy_ = lerpp.tile([P, T * 4], f16, tag="dy", name=f"dy_{tag}")
        dy4 = dy_[:].rearrange("p (t a f) -> p t a f", a=2, f=2)
        nc.vector.tensor_tensor(out=dy4, in0=y1, in1=y0, op=Alu.subtract)
        nc.vector.tensor_tensor(out=dy4, in0=dy4,
                                in1=fy.to_broadcast([P, T, 2, 2]), op=Alu.mult)
        nc.vector.tensor_tensor(out=ay4, in0=dy4, in1=y0, op=Alu.add)

        ayx = ay[:].rearrange("p (t x f) -> p t x f", x=2, f=2)
        x0, x1 = ayx[:, :, 0, :], ayx[:, :, 1, :]
        dx_ = lerpp.tile([P, T * 2], f16, tag="dx", name=f"dx_{tag}")
        dx2 = dx_[:].rearrange("p (t f) -> p t f", f=2)
        nc.vector.tensor_tensor(out=dx2, in0=x1, in1=x0, op=Alu.subtract)
        nc.vector.tensor_tensor(out=dx2, in0=dx2,
                                in1=fx.to_broadcast([P, T, 2]), op=Alu.mult)
        nc.vector.tensor_tensor(out=ol_view, in0=dx2, in1=x0, op=Alu.add)

    def patch_level(nc, coord, coordt, lerpp, l, nodes, xt, i0f5, mega_v, off,
                    ol_view):
        """Patch lerp for level l with `nodes` nodes/dim."""
        R = RES[l]
        ratio = float(R) / float(RC)
        i0f, frach = addr(nc, coord, coordt, xt, R, f"p{l}", itag="i0f_patch")
        # m = floor(i0f5 * ratio) with the same robust f32 math
        mi = coordt.tile([P, T * 3], i32, tag="mi", name=f"mi_{l}")
        nc.vector.tensor_scalar(out=mi[:], in0=i0f5[:], scalar1=ratio,
                                scalar2=None, op0=Alu.mult)
        mf = coordt.tile([P, T * 3], f32, tag="mf", name=f"mf_{l}")
        nc.vector.tensor_copy(out=mf[:], in_=mi[:])
        upm = coordt.tile([P, T * 3], f32, tag="up", name=f"upm_{l}")
        nc.vector.scalar_tensor_tensor(out=upm[:], in0=i0f5[:], scalar=ratio,
                                       in1=mf[:], op0=Alu.mult, op1=Alu.is_lt)
        nc.vector.tensor_tensor(out=mf[:], in0=mf[:], in1=upm[:],
                                op=Alu.subtract)
        b = coordt.tile([P, T * 3], f32, tag="b", name=f"b_{l}")
        nc.vector.tensor_tensor(out=b[:], in0=i0f[:], in1=mf[:], op=Alu.subtract)

        # node weights w_j [P, 3T] f16: shift of [1-f, f] by b (cells = nodes-1)
        # e_j = (b == j) f32; fe_j = frac * e_j;
        # w_0 = e_0 - fe_0; w_j = e_j - fe_j + fe_{j-1}; w_{n-1} = fe_{n-2}
        ncell = nodes - 1
        fracf = coordt.tile([P, T * 3], f32, tag="fracf", name=f"fracf_{l}")
        nc.vector.tensor_copy(out=fracf[:], in_=frach[:])
        es = []
        for j in range(ncell):
            e = coordt.tile([P, T * 3], f32, tag=f"e{j}", name=f"e{j}_{l}")
            nc.vector.tensor_scalar(out=e[:], in0=b[:], scalar1=float(j),
                                    scalar2=None, op0=Alu.is_equal)
            es.append(e)
        fes = []
        for j in range(ncell):
            fe = coordt.tile([P, T * 3], f32, tag=f"fe{j}", name=f"fe{j}_{l}")
            nc.vector.tensor_tensor(out=fe[:], in0=fracf[:], in1=es[j][:],
                                    op=Alu.mult)
            fes.append(fe)
        ws = []
        for j in range(nodes):
            w = coordt.tile([P, T * 3], f16, tag=f"w{j}", name=f"w{j}_{l}")
            if j == 0:
                nc.vector.tensor_tensor(out=w[:], in0=es[0][:], in1=fes[0][:],
                                        op=Alu.subtract)
            elif j < ncell:
                tmp = coordt.tile([P, T * 3], f32, tag="wtmp", name=f"wtmp{j}_{l}")
                nc.vector.tensor_tensor(out=tmp[:], in0=es[j][:], in1=fes[j][:],
                                        op=Alu.subtract)
                nc.vector.tensor_tensor(out=w[:], in0=tmp[:], in1=fes[j - 1][:],
                                        op=Alu.add)
            else:
                nc.vector.tensor_copy(out=w[:], in_=fes[ncell - 1][:])
            ws.append(w)

        def wdim(j, d):
            return ws[j][:].rearrange("p (t c) -> p t c", c=3)[:, :, d]

        n = nodes
        # z reduce -> [P, T, n*n, 2]; (a b) merged so APs stay <= 3 free dims
        pv = mega_v[:, :, off:off + n ** 3 * 2].rearrange(
            "p t (ab c f) -> p t ab c f", ab=n * n, c=n, f=2)
        accz = lerpp.tile([P, T * n * n * 2], f16, tag="accz", name=f"accz_{l}")
        azm = accz[:].rearrange("p (t ab f) -> p t ab f", ab=n * n, f=2)
        tmpz = lerpp.tile([P, T * n * n * 2], f16, tag="tmpz", name=f"tmpz_{l}")
        tzm = tmpz[:].rearrange("p (t ab f) -> p t ab f", ab=n * n, f=2)
        for k in range(n):
            dst = azm if k == 0 else tzm
            nc.vector.tensor_tensor(
                out=dst, in0=pv[:, :, :, k, :],
                in1=wdim(k, 2).to_broadcast([P, T, n * n, 2]), op=Alu.mult)
            if k > 0:
                nc.vector.tensor_tensor(out=accz[:], in0=accz[:], in1=tmpz[:],
                                        op=Alu.add)
        az6 = accz[:].rearrange("p (t a b f) -> p t a b f", a=n, b=n, f=2)
        # y reduce -> [P, T, n, 2]
        accy = lerpp.tile([P, T * n * 2], f16, tag="accy", name=f"accy_{l}")
        ay5 = accy[:].rearrange("p (t a f) -> p t a f", a=n, f=2)
        tmpy = lerpp.tile([P, T * n * 2], f16, tag="tmpy", name=f"tmpy_{l}")
        ty5 = tmpy[:].rearrange("p (t a f) -> p t a f", a=n, f=2)
        for k in range(n):
            dst = ay5 if k == 0 else ty5
            nc.vector.tensor_tensor(
                out=dst, in0=az6[:, :, :, k, :],
                in1=wdim(k, 1).to_broadcast([P, T, n, 2]), op=Alu.mult)
            if k > 0:
                nc.vector.tensor_tensor(out=ay5, in0=ay5, in1=ty5, op=Alu.add)
        # x reduce -> [P, T, 2] into ol_view
        tmpx = lerpp.tile([P, T * 2], f16, tag="tmpx", name=f"tmpx_{l}")
        tx3 = tmpx[:].rearrange("p (t f) -> p t f", f=2)
        for k in range(n):
            dst = ol_view if k == 0 else tx3
            nc.vector.tensor_tensor(
                out=dst, in0=ay5[:, :, k, :],
                in1=wdim(k, 0).to_broadcast([P, T, 2]), op=Alu.mult)
            if k > 0:
                nc.vector.tensor_tensor(out=ol_view, in0=ol_view, in1=tx3,
                                        op=Alu.add)

    with tile.TileContext(nc) as tc:
        with (
            tc.tile_pool(name="xp", bufs=2) as xp,
            tc.tile_pool(name="coord", bufs=2) as coord,
            tc.tile_pool(name="coordt", bufs=1) as coordt,
            tc.tile_pool(name="idxp", bufs=2) as idxp,
            tc.tile_pool(name="megap", bufs=2) as megap,
            tc.tile_pool(name="lerpp", bufs=1) as lerpp,
            tc.tile_pool(name="outp", bufs=2) as outp,
        ):
            def gather_phase(bi):
                xt = xp.tile([P, T * 3], f32, tag="x", name="xt")
                nc.sync.dma_start(out=xt[:],
                                  in_=x_ext.ap()[:, bi * T:(bi + 1) * T, :])
                nc.vector.tensor_scalar(out=xt[:], in0=xt[:], scalar1=XHI,
                                        scalar2=XLO, op0=Alu.min, op1=Alu.max)
                i0f5, frach5 = addr(nc, coord, coordt, xt, RC, "c5")
                i3 = i0f5[:].rearrange("p (t c) -> p t c", c=3)
                ix, iy, iz = i3[:, :, 0], i3[:, :, 1], i3[:, :, 2]
                cellf = coordt.tile([P, T], f32, tag="cellf", name="cellf")
                nc.vector.scalar_tensor_tensor(out=cellf[:], in0=ix,
                                               scalar=float(RC), in1=iy,
                                               op0=Alu.mult, op1=Alu.add)
                idx = idxp.tile([P, T], i32, tag="idx", name="idx")
                nc.vector.scalar_tensor_tensor(out=idx[:], in0=cellf[:],
                                               scalar=float(RC), in1=iz,
                                               op0=Alu.mult, op1=Alu.add)
                mega = megap.tile([P, T * ROW_H], f16, tag="mega", name="mega")
                for j in range(T):
                    nc.gpsimd.indirect_dma_start(
                        out=mega[:, j * ROW_H:(j + 1) * ROW_H],
                        out_offset=None,
                        in_=g_ext.ap(),
                        in_offset=bass.IndirectOffsetOnAxis(
                            ap=idx[:, j:j + 1], axis=0),
                    )
                return bi, xt, i0f5, frach5, mega

            def lerp_phase(st):
                bi, xt, i0f5, frach5, mega = st
                mega_v = mega[:].rearrange("p (t h) -> p t h", h=ROW_H)
                ol = outp.tile([P, T * NUM_LEVELS * FEATS], f16, tag="ol",
                               name="ol")
                ol4 = ol[:].rearrange("p (t l f) -> p t l f", l=NUM_LEVELS,
                                      f=FEATS)
                cube_lerp(nc, lerpp, mega_v[:, :, 0:16], frach5,
                          ol4[:, :, 5, :], "l5")
                i0f1, frach1 = addr(nc, coord, coordt, xt, RES[1], "c1")
                cube_lerp(nc, lerpp, mega_v[:, :, 16:32], frach1,
                          ol4[:, :, 1, :], "l1")
                for l in (0, 2, 3, 4, 6, 7):
                    off, _, nodes = ROW_SPEC[l]
                    patch_level(nc, coord, coordt, lerpp, l, nodes, xt, i0f5,
                                mega_v, off, ol4[:, :, l, :])
                nc.sync.dma_start(
                    out=out_ext.ap()[:, bi * T:(bi + 1) * T, :, :],
                    in_=ol[:],
                )

            pend = None
            for bi in range(n_batches):
                cur = gather_phase(bi)
                if pend is not None:
                    lerp_phase(pend)
                pend = cur
            lerp_phase(pend)
    nc.compile()
    return nc


_PROGRAM_CACHE = {}


def kernel(x01: np.ndarray, tables: np.ndarray, _trace: bool = False,
           _tmpdir: str | None = None) -> np.ndarray:
    from concourse.bass_utils import run_bass_kernel_spmd

    N = x01.shape[0]
    assert N <= N_PAD, (N, N_PAD)

    mega = _build_mega_grid(np.asarray(tables, dtype=np.float32))

    xp = np.zeros((N_PAD, 3), dtype=np.float32)
    xp[:N] = np.asarray(x01, dtype=np.float32)

    key = "prog"
    if key not in _PROGRAM_CACHE:
        _PROGRAM_CACHE[key] = _build_program()
    nc = _PROGRAM_CACHE[key]

    in_maps = []
    for c in range(N_CORES):
        m = {"x": xp[c * PTS_PER_CORE:(c + 1) * PTS_PER_CORE].reshape(P, TP, 3),
             "g": mega}
        in_maps.append(m)

    res = run_bass_kernel_spmd(
        nc, in_maps, core_ids=list(range(N_CORES)),
        trace=_trace, tmpdir=_tmpdir,
    )

    # out per core: [P, TP, 8, 2] fp16 (point-major) -> [N, 16] f32
    parts = [r["out"].reshape(PTS_PER_CORE, NUM_LEVELS * FEATS)
             for r in res.results]
    out = np.concatenate(parts, axis=0).astype(np.float32)
    if _trace:
        kernel.last_exec_time_ns = res.exec_time_ns
        kernel.last_results = res
    return np.ascontiguousarray(out[:N])
